# revision 19
# baseline (speedup 1.0000x reference)
"""Trainium2 Bass kernel for nn_LowRankLayer_dilation (B=4, C=64, H=W=128).

Math: the reference's rank-3 NMF update collapses exactly (all ranks are
initialized identically), and the eps terms are negligible for this input
distribution, giving:

    h   = relu(W_head @ x)            (per-pixel channel matmul)
    g   = W_tail @ h                  (per-pixel channel matmul)
    a   = box9(h)                     (3x3 dilation-2 box sum, edge-clamped)
    n_k = sum_c (a/9)_c * h_c(p+d_k)  (9 taps, d in {-2,0,2}^2)
    out = x + (n_4 / sum_j n_j^2) * sum_k n_k * g(p+d_k)

Sharding: pure data parallel, 8 cores = (batch b, H-half). Each core gets a
68-row halo'd slice packed as 2 channel blocks on 128 partitions:
partition p = c + 64*blk, blk A = slice rows 0..35, blk B = rows 32..67.
Channel reductions/broadcasts run on the PE via block-structured 0/1
matrices. h and g keep 2 replicate-padded columns per side (row stride 132)
so every dilated tap is a strided AP view.

v2 engine assignment (vs v1): the 9 taps are processed in 3 row-groups of 3,
each group's products computed by ONE 4-D-AP DVE op (the dj axis is an
overlapping stride-2 AP dim; av is replicated via a stride-0 broadcast dim).
PSUM->SBUF broadcast copies are split Scalar/GpSimd per 1024-chunk; the box
filter is split DVE/GpSimd; relu runs as tensor_max against a broadcast zero
tile (2x DVE mode) or on Scalar; the residual add reads the bf16 input tile
directly (no separate f32 residual DMA) and the output is bf16. All small
gather/output DMAs issue from the idle sync (SP) queue to keep GpSimd free.
"""
import sys
import contextlib
import numpy as np

sys.path.insert(0, '/opt/trn_rl_repo')

import concourse.bass as bass  # noqa: E402,F401
import concourse.bacc as bacc  # noqa: E402
import concourse.tile as tile  # noqa: E402
import concourse.mybir as mybir  # noqa: E402
from concourse.bass_utils import run_bass_kernel_spmd  # noqa: E402

F32 = mybir.dt.float32
BF16 = mybir.dt.bfloat16
AT = mybir.ActivationFunctionType

N_CORES = 8
RIN = 36          # per-block input rows (with +-2 halo)
ROUT = 32         # per-block output rows
W = 128
WP = W + 4        # padded row stride for h/g
FIN = RIN * W     # 4608
FOUT = ROUT * W   # 4096
HF = 2048         # half (16 out rows) worth of pixels per partition

EDT = BF16


def _build():
    nc = bacc.Bacc("TRN2", target_bir_lowering=False, debug=False,
                   num_devices=N_CORES)
    xb_ext = nc.dram_tensor("xb", [128, FIN], EDT, kind="ExternalInput").ap()
    w2_ext = nc.dram_tensor("w2", [128, 128], EDT, kind="ExternalInput").ap()
    w3_ext = nc.dram_tensor("w3", [128, 128], EDT, kind="ExternalInput").ap()
    bo_ext = nc.dram_tensor("bo", [128, 128], EDT, kind="ExternalInput").ap()
    sb_ext = nc.dram_tensor("sb", [18, 128], EDT, kind="ExternalInput").ap()
    y_ext = nc.dram_tensor("y", [128, FOUT], EDT, kind="ExternalOutput").ap()

    with tile.TileContext(nc) as tc, contextlib.ExitStack() as ctx:
        cpool = ctx.enter_context(tc.tile_pool(name="consts", bufs=1))
        big = ctx.enter_context(tc.tile_pool(name="big", bufs=1))
        gpool = ctx.enter_context(tc.tile_pool(name="grp", bufs=2))
        rows = ctx.enter_context(tc.tile_pool(name="rows", bufs=1))

        w2 = cpool.tile([128, 128], EDT)
        nc.sync.dma_start(w2[:], w2_ext[:])
        w3 = cpool.tile([128, 128], EDT)
        nc.sync.dma_start(w3[:], w3_ext[:])
        xbt = big.tile([128, FIN], EDT)
        for c0, c1 in ((0, 2048), (2048, 4096), (4096, 4608)):
            nc.sync.dma_start(xbt[:, c0:c1], xb_ext[:, c0:c1])
        bo = cpool.tile([128, 128], EDT)
        nc.gpsimd.dma_start(bo[:], bo_ext[:])
        sbm = cpool.tile([18, 128], EDT)
        nc.gpsimd.dma_start(sbm[:], sb_ext[:])

        xb3 = xbt.rearrange("p (r w) -> p r w", w=W)

        # h/g: (RIN, WP) row layout; data at cols 2..129, replicate pads at
        # cols 0,1,130,131.
        hf = big.tile([128, RIN * WP], EDT)
        gf = big.tile([128, RIN * WP], EDT)
        h3 = hf.rearrange("p (r w) -> p r w", w=WP)
        g3 = gf.rearrange("p (r w) -> p r w", w=WP)

        def tap(t3, di, dj, nrows=ROUT, r0=2):
            rr = r0 + di
            return t3[:, rr:rr + nrows, 2 + dj:2 + dj + W]

        def tap4(t3, di, rh):
            """All three dj taps of row-group di as one overlapping 4-D view:
            [128, 3(dj), 16, W]; dj axis = stride-2 cols of the padded rows."""
            v = t3[:, 2 + di + rh:2 + di + rh + 16, 0:W].unsqueeze(1)
            v.ap[1] = [2, 3]
            return v

        PADS = ((0, 2), (1, 2), (130, 129), (131, 129))

        # ---- head + tail matmuls: h = relu(W_head @ x), g = W_tail @ h ----
        with tc.tile_pool(name="psmm", bufs=2, space="PSUM") as psmm:
            for j in range(2):
                ps = psmm.tile([128, 2048], F32, tag="mm")
                for q in range(4):
                    c0 = j * 2048 + q * 512
                    nc.tensor.matmul(ps[:, q * 512:(q + 1) * 512], w2[:],
                                     xbt[:, c0:c0 + 512], start=True, stop=True)
                nc.scalar.activation(h3[:, j * 16:j * 16 + 16, 2:2 + W],
                                     ps[:].rearrange("p (r w) -> p r w", w=W),
                                     AT.Relu)
            # pads for rows 0..31 (DVE, cheap) unblock the box row pass
            for dst_c, src_c in PADS:
                nc.vector.tensor_copy(h3[:, 0:32, dst_c:dst_c + 1],
                                      h3[:, 0:32, src_c:src_c + 1])
            ps = psmm.tile([128, 2048], F32, tag="mm")
            nc.tensor.matmul(ps[:, 0:512], w2[:], xbt[:, 4096:4608],
                             start=True, stop=True)
            nc.scalar.activation(h3[:, 32:36, 2:2 + W],
                                 ps[:, 0:512].rearrange("p (r w) -> p r w", w=W),
                                 AT.Relu)
            for dst_c, src_c in PADS:
                nc.vector.tensor_copy(h3[:, 32:36, dst_c:dst_c + 1],
                                      h3[:, 32:36, src_c:src_c + 1])

            # box filter row pass (all DVE): T rows 0..19 first (gates av
            # half 0), then rows 20..35
            T = big.tile([128, FIN], EDT)
            T3 = T.rearrange("p (r w) -> p r w", w=W)
            nc.vector.tensor_add(T3[:, 0:20, :], tap(h3, -2, -2, 20, 2),
                                 tap(h3, -2, 0, 20, 2))
            nc.vector.tensor_add(T3[:, 0:20, :], T3[:, 0:20, :],
                                 tap(h3, -2, 2, 20, 2))
            # col pass half 0 on DVE -> av ready for the first prod group
            av = big.tile([128, FOUT], EDT)
            nc.vector.tensor_add(av[:, 0:HF], T[:, 0:HF],
                                 T[:, 2 * W:2 * W + HF])
            nc.vector.tensor_add(av[:, 0:HF], av[:, 0:HF],
                                 T[:, 4 * W:4 * W + HF])
            nc.vector.tensor_add(T3[:, 20:36, :], tap(h3, -2, -2, 16, 22),
                                 tap(h3, -2, 0, 16, 22))
            nc.vector.tensor_add(T3[:, 20:36, :], T3[:, 20:36, :],
                                 tap(h3, -2, 2, 16, 22))
            # col pass half 1 on Pool (needed only once half 1 starts)
            nc.gpsimd.tensor_add(av[:, HF:2 * HF], T[:, HF:2 * HF],
                                 T[:, HF + 2 * W:HF + 2 * W + HF])
            nc.gpsimd.tensor_add(av[:, HF:2 * HF], av[:, HF:2 * HF],
                                 T[:, HF + 4 * W:HF + 4 * W + HF])

            for j in range(2):
                ps = psmm.tile([128, 2048], F32, tag="mm")
                for q in range(4):
                    r0 = j * 16 + q * 4
                    nc.tensor.matmul(
                        ps[:, q * 512:(q + 1) * 512], w3[:],
                        h3[:, r0:r0 + 4, 2:2 + W], start=True, stop=True)
                nc.scalar.copy(g3[:, j * 16:j * 16 + 16, 2:2 + W],
                               ps[:].rearrange("p (r w) -> p r w", w=W))
            ps = psmm.tile([128, 2048], F32, tag="mm")
            nc.tensor.matmul(ps[:, 0:512], w3[:], h3[:, 32:36, 2:2 + W],
                             start=True, stop=True)
            nc.scalar.copy(g3[:, 32:36, 2:2 + W],
                           ps[:, 0:512].rearrange("p (r w) -> p r w", w=W))

        av3 = av.rearrange("p (r w) -> p r w", w=W)
        for dst_c, src_c in PADS:
            nc.vector.tensor_copy(g3[:, :, dst_c:dst_c + 1],
                                  g3[:, :, src_c:src_c + 1])

        # ---- k loop: groups of 3 taps (same row offset di, three dj) ----
        nst = cpool.tile([18, FOUT], EDT)       # n_k rows, row pair by kr
        nsq = cpool.tile([18, FOUT], EDT)
        facc = big.tile([128, FOUT], EDT)

        with tc.tile_pool(name="psnk", bufs=2, space="PSUM") as psnk:

            def cf_steps(half, nb_c):
                """Deferred tail for one half: N2 (broadcast to all 128 rows
                via the sbm matmul), reciprocal, cf = nb_center*rcp, residual,
                DMA out. Fed at group boundaries of the NEXT half so every
                engine keeps independent ready work. nb_c = the center-tap
                broadcast slice of nb3 (group 1, j=1), kept alive by bufs=3."""
                hs = slice(half * HF, (half + 1) * HF)
                nc.scalar.square(nsq[:, hs], nst[:, hs])
                s2ps = psnk.tile([128, HF], F32, tag="nk")
                for q in range(4):
                    c0 = half * HF + q * 512
                    nc.tensor.matmul(s2ps[:, q * 512:(q + 1) * 512], sbm[:],
                                     nsq[:, c0:c0 + 512],
                                     start=True, stop=True)
                rcp = rows.tile([128, HF], F32, tag="rcp")
                nc.vector.reciprocal_approx_fast(rcp[:], s2ps[:])
                yield
                cfb = rows.tile([128, HF], EDT, tag="cfb")
                nc.vector.tensor_mul(cfb[:], nb_c, rcp[:])
                yield
                res = rows.tile([128, HF], EDT, tag="res")
                res3 = res.rearrange("p (r w) -> p r w", w=W)
                eng = nc.gpsimd if half == 0 else nc.vector
                eng.tensor_mul(res[:], facc[:, hs], cfb[:])
                eng.tensor_add(res3[:], res3[:],
                               xb3[:, 2 + half * 16:2 + half * 16 + 16, :])
                nc.sync.dma_start(y_ext[:, hs], res[:])
                yield

            pending = None
            for half in range(2):
                rh = half * 16
                hs = slice(half * HF, (half + 1) * HF)

                def emit_prod3(grp):
                    di = 2 * grp - 2
                    prod3 = gpool.tile([128, 3 * HF], EDT, tag="prod3")
                    o4 = prod3.rearrange("p (a r w) -> p a r w", a=3, w=W)
                    in0 = av3[:, rh:rh + 16, :].unsqueeze(1) \
                        .broadcast_to([128, 3, 16, W])
                    nc.vector.tensor_mul(o4, in0, tap4(h3, di, rh))
                    return prod3

                prods = {0: emit_prod3(0)}
                pk3s = {}

                def emit_facc(grp):
                    """facc[:, hs] (+)= the 3 pk slices of group grp, via
                    gpsimd software-DGE accumulate DMAs (k==0 initializes)."""
                    pk3 = pk3s.pop(grp)
                    for j in range(3):
                        k = 3 * grp + j
                        op = (mybir.AluOpType.bypass if k == 0
                              else mybir.AluOpType.add)
                        nc.gpsimd.dma_start(facc[:, hs],
                                            pk3[:, j * HF:(j + 1) * HF],
                                            accum_op=op)

                for grp in range(3):
                    di = 2 * grp - 2
                    prod3 = prods.pop(grp)
                    nb3 = gpool.tile([128, 3 * HF], EDT, tag="nb3", bufs=3)
                    if grp == 1:
                        nb_center = nb3[:, HF:2 * HF]
                    for j in range(3):
                        k = 3 * grp + j
                        pst = psnk.tile([128, 2048], F32, tag="nk")
                        for q in range(4):
                            c0 = j * HF + q * 512
                            nc.tensor.matmul(
                                pst[:, q * 512:(q + 1) * 512], bo[:],
                                prod3[:, c0:c0 + 512],
                                start=True, stop=True)
                        nc.scalar.copy(nb3[:, j * HF:(j + 1) * HF], pst[:])
                        kr = (k - 4) % 9      # put k=4 (center) at rows 0..1
                        nc.sync.dma_start(nst[2 * kr:2 * kr + 1, hs],
                                          nb3[0:1, j * HF:(j + 1) * HF])
                        nc.sync.dma_start(nst[2 * kr + 1:2 * kr + 2, hs],
                                          nb3[64:65, j * HF:(j + 1) * HF])

                    if grp + 1 < 3:
                        prods[grp + 1] = emit_prod3(grp + 1)

                    pk3 = gpool.tile([128, 3 * HF], EDT, tag="pk3")
                    o4 = pk3.rearrange("p (a r w) -> p a r w", a=3, w=W)
                    i0 = nb3.rearrange("p (a r w) -> p a r w", a=3, w=W)
                    nc.vector.tensor_mul(o4, i0, tap4(g3, di, rh))
                    pk3s[grp] = pk3
                    emit_facc(grp)
                    if pending is not None:
                        next(pending, None)
                if pending is not None:
                    for _ in pending:
                        pass
                pending = cf_steps(half, nb_center)
                next(pending, None)
            for _ in pending:
                pass

    nc.compile()
    return nc


_NC_CACHE = [None]


def _get_nc():
    if _NC_CACHE[0] is None:
        _NC_CACHE[0] = _build()
    return _NC_CACHE[0]


def _host_prep(x):
    import ml_dtypes
    B, Cc, H, Ww = x.shape
    in_maps = []
    for core in range(N_CORES):
        b, half = core // 2, core % 2
        r0 = 64 * half
        gidx = np.clip(np.arange(r0 - 2, r0 + 66), 0, H - 1)
        xs = x[b][:, gidx, :]                     # (64, 68, 128)
        packed = np.ascontiguousarray(
            np.concatenate([xs[:, 0:36], xs[:, 32:68]], axis=0))
        in_maps.append({
            "xb": packed.reshape(128, FIN).astype(ml_dtypes.bfloat16),
        })
    return in_maps


def _const_maps(W_head, W_tail):
    import ml_dtypes

    def to_edt(a):
        return a.astype(ml_dtypes.bfloat16) if EDT == BF16 else a.astype(np.float32)

    w2 = np.zeros((128, 128), np.float32)
    w2[:64, :64] = W_head.T
    w2[64:, 64:] = W_head.T
    w3 = np.zeros((128, 128), np.float32)
    w3[:64, :64] = W_tail.T
    w3[64:, 64:] = W_tail.T
    bo = np.zeros((128, 128), np.float32)
    bo[:64, :64] = 1.0 / 9.0
    bo[64:, 64:] = 1.0 / 9.0
    sb = np.zeros((18, 128), np.float32)
    sb[0::2, :64] = 1.0
    sb[1::2, 64:] = 1.0
    return {"w2": to_edt(w2), "w3": to_edt(w3), "bo": to_edt(bo),
            "sb": to_edt(sb)}


def kernel(x, W_head, W_tail):
    x = np.asarray(x, np.float32)
    W_head = np.asarray(W_head, np.float32)
    W_tail = np.asarray(W_tail, np.float32)
    nc = _get_nc()
    consts = _const_maps(W_head, W_tail)
    in_maps = [{**m, **consts} for m in _host_prep(x)]
    res = run_bass_kernel_spmd(nc, in_maps, list(range(N_CORES)))
    out = np.empty_like(x)
    for core in range(N_CORES):
        b, half = core // 2, core % 2
        r0 = 64 * half
        y = res.results[core]["y"].astype(np.float32).reshape(128, ROUT, W)
        out[b, :, r0:r0 + 32, :] = y[:64]
        out[b, :, r0 + 32:r0 + 64, :] = y[64:]
    return out


# revision 25
# speedup vs baseline: 1.1415x; 1.1415x over previous
"""Trainium2 Bass kernel for nn_LowRankLayer_dilation (B=4, C=64, H=W=128).

Math: the reference's rank-3 NMF update collapses exactly (all ranks are
initialized identically), and the eps terms are negligible for this input
distribution, giving:

    h   = relu(W_head @ x)            (per-pixel channel matmul)
    g   = W_tail @ h                  (per-pixel channel matmul)
    a   = box9(h)                     (3x3 dilation-2 box sum, edge-clamped)
    n_k = sum_c (a/9)_c * h_c(p+d_k)  (9 taps, d in {-2,0,2}^2)
    out = x + (n_4 / sum_j n_j^2) * sum_k n_k * g(p+d_k)

Sharding: pure data parallel, 8 cores = (batch b, H-half). Each core gets a
68-row halo'd slice packed as 2 channel blocks on 128 partitions:
partition p = c + 64*blk, blk A = slice rows 0..35, blk B = rows 32..67.
Channel reductions/broadcasts run on the PE via block-structured 0/1
matrices. h and g keep 2 replicate-padded columns per side (row stride 132)
so every dilated tap is a strided AP view.

v2 engine assignment (vs v1): the 9 taps are processed in 3 row-groups of 3,
each group's products computed by ONE 4-D-AP DVE op (the dj axis is an
overlapping stride-2 AP dim; av is replicated via a stride-0 broadcast dim).
PSUM->SBUF broadcast copies are split Scalar/GpSimd per 1024-chunk; the box
filter is split DVE/GpSimd; relu runs as tensor_max against a broadcast zero
tile (2x DVE mode) or on Scalar; the residual add reads the bf16 input tile
directly (no separate f32 residual DMA) and the output is bf16. All small
gather/output DMAs issue from the idle sync (SP) queue to keep GpSimd free.
"""
import sys
import contextlib
import numpy as np

sys.path.insert(0, '/opt/trn_rl_repo')

import concourse.bass as bass  # noqa: E402,F401
import concourse.bacc as bacc  # noqa: E402
import concourse.tile as tile  # noqa: E402
import concourse.mybir as mybir  # noqa: E402
from concourse.bass_utils import run_bass_kernel_spmd  # noqa: E402

F32 = mybir.dt.float32
BF16 = mybir.dt.bfloat16
AT = mybir.ActivationFunctionType

N_CORES = 8
RIN = 36          # per-block input rows (with +-2 halo)
ROUT = 32         # per-block output rows
W = 128
WP = W + 4        # padded row stride for h/g
FIN = RIN * W     # 4608
FOUT = ROUT * W   # 4096
HF = 2048         # half (16 out rows) worth of pixels per partition

EDT = BF16


def _build():
    nc = bacc.Bacc("TRN2", target_bir_lowering=False, debug=False,
                   num_devices=N_CORES)
    xb_ext = nc.dram_tensor("xb", [128, FIN], EDT, kind="ExternalInput").ap()
    w2_ext = nc.dram_tensor("w2", [128, 128], EDT, kind="ExternalInput").ap()
    w3_ext = nc.dram_tensor("w3", [128, 128], EDT, kind="ExternalInput").ap()
    bo_ext = nc.dram_tensor("bo", [128, 128], EDT, kind="ExternalInput").ap()
    sb_ext = nc.dram_tensor("sb", [18, 128], EDT, kind="ExternalInput").ap()
    id_ext = nc.dram_tensor("idm", [128, 128], EDT, kind="ExternalInput").ap()
    y_ext = nc.dram_tensor("y", [128, FOUT], EDT, kind="ExternalOutput").ap()

    with tile.TileContext(nc) as tc, contextlib.ExitStack() as ctx:
        cpool = ctx.enter_context(tc.tile_pool(name="consts", bufs=1))
        big = ctx.enter_context(tc.tile_pool(name="big", bufs=1))
        gpool = ctx.enter_context(tc.tile_pool(name="grp", bufs=2))
        rows = ctx.enter_context(tc.tile_pool(name="rows", bufs=1))

        w2 = cpool.tile([128, 128], EDT)
        nc.sync.dma_start(w2[:], w2_ext[:])
        w3 = cpool.tile([128, 128], EDT)
        nc.sync.dma_start(w3[:], w3_ext[:])
        xbt = big.tile([128, FIN], EDT)
        for c0, c1 in ((0, 2048), (2048, 4096), (4096, 4608)):
            nc.sync.dma_start(xbt[:, c0:c1], xb_ext[:, c0:c1])
        bo = cpool.tile([128, 128], EDT)
        nc.gpsimd.dma_start(bo[:], bo_ext[:])
        sbm = cpool.tile([18, 128], EDT)
        nc.gpsimd.dma_start(sbm[:], sb_ext[:])
        idm = cpool.tile([128, 128], EDT)
        nc.gpsimd.dma_start(idm[:], id_ext[:])

        xb3 = xbt.rearrange("p (r w) -> p r w", w=W)

        # h/g: (RIN, WP) row layout; data at cols 2..129, replicate pads at
        # cols 0,1,130,131.
        hf = big.tile([128, RIN * WP], EDT)
        gf = big.tile([128, RIN * WP], EDT)
        h3 = hf.rearrange("p (r w) -> p r w", w=WP)
        g3 = gf.rearrange("p (r w) -> p r w", w=WP)

        def tap(t3, di, dj, nrows=ROUT, r0=2):
            rr = r0 + di
            return t3[:, rr:rr + nrows, 2 + dj:2 + dj + W]

        def tap4(t3, di, rh):
            """All three dj taps of row-group di as one overlapping 4-D view:
            [128, 3(dj), 16, W]; dj axis = stride-2 cols of the padded rows."""
            v = t3[:, 2 + di + rh:2 + di + rh + 16, 0:W].unsqueeze(1)
            v.ap[1] = [2, 3]
            return v

        PADS = ((0, 2), (1, 2), (130, 129), (131, 129))

        # ---- head + tail matmuls: h = relu(W_head @ x), g = W_tail @ h ----
        with tc.tile_pool(name="psmm", bufs=2, space="PSUM") as psmm:
            for j in range(2):
                ps = psmm.tile([128, 2048], F32, tag="mm")
                for q in range(4):
                    c0 = j * 2048 + q * 512
                    nc.tensor.matmul(ps[:, q * 512:(q + 1) * 512], w2[:],
                                     xbt[:, c0:c0 + 512], start=True, stop=True)
                nc.scalar.activation(h3[:, j * 16:j * 16 + 16, 2:2 + W],
                                     ps[:].rearrange("p (r w) -> p r w", w=W),
                                     AT.Relu)
            # pads for rows 0..31 (Scalar, sequential with relu) unblock the
            # box row pass
            for dst_c, src_c in PADS:
                nc.scalar.copy(h3[:, 0:32, dst_c:dst_c + 1],
                               h3[:, 0:32, src_c:src_c + 1])
            ps = psmm.tile([128, 2048], F32, tag="mm")
            nc.tensor.matmul(ps[:, 0:512], w2[:], xbt[:, 4096:4608],
                             start=True, stop=True)
            nc.scalar.activation(h3[:, 32:36, 2:2 + W],
                                 ps[:, 0:512].rearrange("p (r w) -> p r w", w=W),
                                 AT.Relu)
            for dst_c, src_c in PADS:
                nc.scalar.copy(h3[:, 32:36, dst_c:dst_c + 1],
                               h3[:, 32:36, src_c:src_c + 1])

            # box filter row pass (all DVE): T rows 0..19 first (gates av
            # half 0), then rows 20..35
            T = big.tile([128, FIN], EDT)
            T3 = T.rearrange("p (r w) -> p r w", w=W)
            nc.vector.tensor_add(T3[:, 0:20, :], tap(h3, -2, -2, 20, 2),
                                 tap(h3, -2, 0, 20, 2))
            nc.vector.tensor_add(T3[:, 0:20, :], T3[:, 0:20, :],
                                 tap(h3, -2, 2, 20, 2))
            # col pass half 0 on DVE -> av ready for the first prod group
            av = big.tile([128, FOUT], EDT)
            nc.vector.tensor_add(av[:, 0:HF], T[:, 0:HF],
                                 T[:, 2 * W:2 * W + HF])
            nc.vector.tensor_add(av[:, 0:HF], av[:, 0:HF],
                                 T[:, 4 * W:4 * W + HF])
            nc.vector.tensor_add(T3[:, 20:36, :], tap(h3, -2, -2, 16, 22),
                                 tap(h3, -2, 0, 16, 22))
            nc.vector.tensor_add(T3[:, 20:36, :], T3[:, 20:36, :],
                                 tap(h3, -2, 2, 16, 22))
            # col pass half 1 on Pool (needed only once half 1 starts)
            nc.gpsimd.tensor_add(av[:, HF:2 * HF], T[:, HF:2 * HF],
                                 T[:, HF + 2 * W:HF + 2 * W + HF])
            nc.gpsimd.tensor_add(av[:, HF:2 * HF], av[:, HF:2 * HF],
                                 T[:, HF + 4 * W:HF + 4 * W + HF])

            for j in range(2):
                ps = psmm.tile([128, 2048], F32, tag="mm")
                for q in range(4):
                    r0 = j * 16 + q * 4
                    nc.tensor.matmul(
                        ps[:, q * 512:(q + 1) * 512], w3[:],
                        h3[:, r0:r0 + 4, 2:2 + W], start=True, stop=True)
                nc.scalar.copy(g3[:, j * 16:j * 16 + 16, 2:2 + W],
                               ps[:].rearrange("p (r w) -> p r w", w=W))
            ps = psmm.tile([128, 2048], F32, tag="mm")
            nc.tensor.matmul(ps[:, 0:512], w3[:], h3[:, 32:36, 2:2 + W],
                             start=True, stop=True)
            nc.scalar.copy(g3[:, 32:36, 2:2 + W],
                           ps[:, 0:512].rearrange("p (r w) -> p r w", w=W))

        av3 = av.rearrange("p (r w) -> p r w", w=W)
        for dst_c, src_c in PADS:
            nc.scalar.copy(g3[:, :, dst_c:dst_c + 1],
                           g3[:, :, src_c:src_c + 1])

        # ---- k loop: groups of 3 taps (same row offset di, three dj) ----
        nst = cpool.tile([18, FOUT], EDT)       # n_k rows, row pair by kr
        nsq = cpool.tile([18, FOUT], EDT)
        facc = big.tile([128, FOUT], EDT)

        with tc.tile_pool(name="psnk", bufs=2, space="PSUM") as psnk, \
                tc.tile_pool(name="psfa", bufs=1, space="PSUM") as psfa:

            def cf_steps(half, nb_c):
                """Deferred tail for one half: N2 (broadcast to all 128 rows
                via the sbm matmul), reciprocal, cf = nb_center*rcp, residual,
                DMA out. Fed at group boundaries of the NEXT half so every
                engine keeps independent ready work. nb_c = the center-tap
                broadcast slice of nb3 (group 1, j=1), kept alive by bufs=3."""
                hs = slice(half * HF, (half + 1) * HF)
                nc.scalar.square(nsq[:, hs], nst[:, hs])
                s2ps = psfa.tile([128, HF], F32, tag="ps")
                for q in range(4):
                    c0 = half * HF + q * 512
                    nc.tensor.matmul(s2ps[:, q * 512:(q + 1) * 512], sbm[:],
                                     nsq[:, c0:c0 + 512],
                                     start=True, stop=True)
                rcp = rows.tile([128, HF], F32, tag="rcp")
                nc.vector.reciprocal_approx_fast(rcp[:], s2ps[:])
                yield
                cfb = rows.tile([128, HF], EDT, tag="cfb")
                eng = nc.gpsimd if half == 0 else nc.vector
                eng.tensor_mul(cfb[:], nb_c, rcp[:])
                yield
                res = rows.tile([128, HF], EDT, tag="res")
                nc.vector.tensor_mul(res[:], facc[:, hs], cfb[:])
                nc.vector.tensor_add(res[:], res[:],
                                     xbt[:, 2 * W + half * HF:
                                          2 * W + half * HF + HF])
                nc.sync.dma_start(y_ext[:, hs], res[:])
                yield

            pending = None
            for half in range(2):
                rh = half * 16
                hs = slice(half * HF, (half + 1) * HF)
                facc_ps = psfa.tile([128, HF], F32, tag="ps")

                def emit_prod3(grp):
                    di = 2 * grp - 2
                    prod3 = gpool.tile([128, 3 * HF], EDT, tag="prod3")
                    o4 = prod3.rearrange("p (a r w) -> p a r w", a=3, w=W)
                    in0 = av3[:, rh:rh + 16, :].unsqueeze(1) \
                        .broadcast_to([128, 3, 16, W])
                    nc.vector.tensor_mul(o4, in0, tap4(h3, di, rh))
                    return prod3

                prods = {0: emit_prod3(0)}
                pk3s = {}

                def emit_ident(grp):
                    pk3 = pk3s.pop(grp)
                    for j in range(3):
                        k = 3 * grp + j
                        for q in range(4):
                            c0 = j * HF + q * 512
                            nc.tensor.matmul(
                                facc_ps[:, q * 512:(q + 1) * 512], idm[:],
                                pk3[:, c0:c0 + 512],
                                start=(k == 0), stop=(k == 8))

                for grp in range(3):
                    di = 2 * grp - 2
                    prod3 = prods.pop(grp)
                    nb3 = gpool.tile([128, 3 * HF], EDT, tag="nb3", bufs=3)
                    if grp == 1:
                        nb_center = nb3[:, HF:2 * HF]
                    for j in range(3):
                        k = 3 * grp + j
                        for ch in range(2):
                            pst = psnk.tile([128, 1024], F32, tag="nk")
                            for q in range(2):
                                c0 = j * HF + ch * 1024 + q * 512
                                nc.tensor.matmul(
                                    pst[:, q * 512:(q + 1) * 512], bo[:],
                                    prod3[:, c0:c0 + 512],
                                    start=True, stop=True)
                            nc.scalar.copy(
                                nb3[:, j * HF + ch * 1024:
                                    j * HF + (ch + 1) * 1024], pst[:])
                        kr = (k - 4) % 9      # put k=4 (center) at rows 0..1
                        nc.sync.dma_start(nst[2 * kr:2 * kr + 1, hs],
                                          nb3[0:1, j * HF:(j + 1) * HF])
                        nc.sync.dma_start(nst[2 * kr + 1:2 * kr + 2, hs],
                                          nb3[64:65, j * HF:(j + 1) * HF])

                    if grp + 1 < 3:
                        prods[grp + 1] = emit_prod3(grp + 1)

                    pk3 = gpool.tile([128, 3 * HF], EDT, tag="pk3")
                    o4 = pk3.rearrange("p (a r w) -> p a r w", a=3, w=W)
                    i0 = nb3.rearrange("p (a r w) -> p a r w", a=3, w=W)
                    nc.vector.tensor_mul(o4, i0, tap4(g3, di, rh))
                    pk3s[grp] = pk3
                    if grp >= 1:
                        emit_ident(grp - 1)
                    if pending is not None:
                        next(pending, None)
                emit_ident(2)
                nc.scalar.copy(facc[:, hs], facc_ps[:])
                if pending is not None:
                    for _ in pending:
                        pass
                pending = cf_steps(half, nb_center)
                next(pending, None)
            for _ in pending:
                pass

    nc.compile()
    return nc


_NC_CACHE = [None]


def _get_nc():
    if _NC_CACHE[0] is None:
        _NC_CACHE[0] = _build()
    return _NC_CACHE[0]


def _host_prep(x):
    import ml_dtypes
    B, Cc, H, Ww = x.shape
    in_maps = []
    for core in range(N_CORES):
        b, half = core // 2, core % 2
        r0 = 64 * half
        gidx = np.clip(np.arange(r0 - 2, r0 + 66), 0, H - 1)
        xs = x[b][:, gidx, :]                     # (64, 68, 128)
        packed = np.ascontiguousarray(
            np.concatenate([xs[:, 0:36], xs[:, 32:68]], axis=0))
        in_maps.append({
            "xb": packed.reshape(128, FIN).astype(ml_dtypes.bfloat16),
        })
    return in_maps


def _const_maps(W_head, W_tail):
    import ml_dtypes

    def to_edt(a):
        return a.astype(ml_dtypes.bfloat16) if EDT == BF16 else a.astype(np.float32)

    w2 = np.zeros((128, 128), np.float32)
    w2[:64, :64] = W_head.T
    w2[64:, 64:] = W_head.T
    w3 = np.zeros((128, 128), np.float32)
    w3[:64, :64] = W_tail.T
    w3[64:, 64:] = W_tail.T
    bo = np.zeros((128, 128), np.float32)
    bo[:64, :64] = 1.0 / 9.0
    bo[64:, 64:] = 1.0 / 9.0
    sb = np.zeros((18, 128), np.float32)
    sb[0::2, :64] = 1.0
    sb[1::2, 64:] = 1.0
    return {"w2": to_edt(w2), "w3": to_edt(w3), "bo": to_edt(bo),
            "sb": to_edt(sb),
            "idm": to_edt(np.eye(128, dtype=np.float32))}


def kernel(x, W_head, W_tail):
    x = np.asarray(x, np.float32)
    W_head = np.asarray(W_head, np.float32)
    W_tail = np.asarray(W_tail, np.float32)
    nc = _get_nc()
    consts = _const_maps(W_head, W_tail)
    in_maps = [{**m, **consts} for m in _host_prep(x)]
    res = run_bass_kernel_spmd(nc, in_maps, list(range(N_CORES)))
    out = np.empty_like(x)
    for core in range(N_CORES):
        b, half = core // 2, core % 2
        r0 = 64 * half
        y = res.results[core]["y"].astype(np.float32).reshape(128, ROUT, W)
        out[b, :, r0:r0 + 32, :] = y[:64]
        out[b, :, r0 + 32:r0 + 64, :] = y[64:]
    return out


# revision 27
# speedup vs baseline: 1.2246x; 1.0729x over previous
"""Trainium2 Bass kernel for nn_LowRankLayer_dilation (B=4, C=64, H=W=128).

Math: the reference's rank-3 NMF update collapses exactly (all ranks are
initialized identically), and the eps terms are negligible for this input
distribution, giving:

    h   = relu(W_head @ x)            (per-pixel channel matmul)
    g   = W_tail @ h                  (per-pixel channel matmul)
    a   = box9(h)                     (3x3 dilation-2 box sum, edge-clamped)
    n_k = sum_c (a/9)_c * h_c(p+d_k)  (9 taps, d in {-2,0,2}^2)
    out = x + (n_4 / sum_j n_j^2) * sum_k n_k * g(p+d_k)

Sharding: pure data parallel, 8 cores = (batch b, H-half). Each core gets a
68-row halo'd slice packed as 2 channel blocks on 128 partitions:
partition p = c + 64*blk, blk A = slice rows 0..35, blk B = rows 32..67.
Channel reductions/broadcasts run on the PE via block-structured 0/1
matrices. h and g keep 2 replicate-padded columns per side (row stride 132)
so every dilated tap is a strided AP view.

v2 engine assignment (vs v1): the 9 taps are processed in 3 row-groups of 3,
each group's products computed by ONE 4-D-AP DVE op (the dj axis is an
overlapping stride-2 AP dim; av is replicated via a stride-0 broadcast dim).
PSUM->SBUF broadcast copies are split Scalar/GpSimd per 1024-chunk; the box
filter is split DVE/GpSimd; relu runs as tensor_max against a broadcast zero
tile (2x DVE mode) or on Scalar; the residual add reads the bf16 input tile
directly (no separate f32 residual DMA) and the output is bf16. All small
gather/output DMAs issue from the idle sync (SP) queue to keep GpSimd free.
"""
import sys
import contextlib
import numpy as np

sys.path.insert(0, '/opt/trn_rl_repo')

import concourse.bass as bass  # noqa: E402,F401
import concourse.bacc as bacc  # noqa: E402
import concourse.tile as tile  # noqa: E402
import concourse.mybir as mybir  # noqa: E402
from concourse.bass_utils import run_bass_kernel_spmd  # noqa: E402

F32 = mybir.dt.float32
BF16 = mybir.dt.bfloat16
AT = mybir.ActivationFunctionType

N_CORES = 8
RIN = 36          # per-block input rows (with +-2 halo)
ROUT = 32         # per-block output rows
W = 128
WP = W + 4        # padded row stride for h/g
FIN = RIN * W     # 4608
FOUT = ROUT * W   # 4096
HF = 2048         # half (16 out rows) worth of pixels per partition

EDT = BF16


def _build():
    nc = bacc.Bacc("TRN2", target_bir_lowering=False, debug=False,
                   num_devices=N_CORES)
    xb_ext = nc.dram_tensor("xb", [128, FIN], EDT, kind="ExternalInput").ap()
    w2_ext = nc.dram_tensor("w2", [128, 128], EDT, kind="ExternalInput").ap()
    w3_ext = nc.dram_tensor("w3", [128, 128], EDT, kind="ExternalInput").ap()
    bo_ext = nc.dram_tensor("bo", [128, 128], EDT, kind="ExternalInput").ap()
    sb_ext = nc.dram_tensor("sb", [18, 128], EDT, kind="ExternalInput").ap()
    id_ext = nc.dram_tensor("idm", [128, 128], EDT, kind="ExternalInput").ap()
    y_ext = nc.dram_tensor("y", [128, FOUT], EDT, kind="ExternalOutput").ap()

    with tile.TileContext(nc) as tc, contextlib.ExitStack() as ctx:
        cpool = ctx.enter_context(tc.tile_pool(name="consts", bufs=1))
        big = ctx.enter_context(tc.tile_pool(name="big", bufs=1))
        gpool = ctx.enter_context(tc.tile_pool(name="grp", bufs=2))
        rows = ctx.enter_context(tc.tile_pool(name="rows", bufs=2))

        # startup: weights first (gate LDW), x chunks spread over the three
        # HWDGE queues so transfers run in parallel
        w2 = cpool.tile([128, 128], EDT)
        nc.sync.dma_start(w2[:], w2_ext[:])
        xbt = big.tile([128, FIN], EDT)
        nc.sync.dma_start(xbt[:, 0:2048], xb_ext[:, 0:2048])
        nc.scalar.dma_start(xbt[:, 2048:4096], xb_ext[:, 2048:4096])
        nc.gpsimd.dma_start(xbt[:, 4096:4608], xb_ext[:, 4096:4608])
        w3 = cpool.tile([128, 128], EDT)
        nc.sync.dma_start(w3[:], w3_ext[:])
        bo = cpool.tile([128, 128], EDT)
        nc.gpsimd.dma_start(bo[:], bo_ext[:])
        sbm = cpool.tile([18, 128], EDT)
        nc.gpsimd.dma_start(sbm[:], sb_ext[:])
        idm = cpool.tile([128, 128], EDT)
        nc.gpsimd.dma_start(idm[:], id_ext[:])
        zb = cpool.tile([128, 512], EDT)
        nc.gpsimd.memset(zb[:], 0.0)

        # h/g: (RIN, WP) row layout; data at cols 2..129, replicate pads at
        # cols 0,1,130,131.
        hf = big.tile([128, RIN * WP], EDT)
        gf = big.tile([128, RIN * WP], EDT)
        h3 = hf.rearrange("p (r w) -> p r w", w=WP)
        g3 = gf.rearrange("p (r w) -> p r w", w=WP)

        def tap(t3, di, dj, nrows=ROUT, r0=2):
            rr = r0 + di
            return t3[:, rr:rr + nrows, 2 + dj:2 + dj + W]

        PADS = ((0, 2), (1, 2), (130, 129), (131, 129))

        # ---- head + tail matmuls: h = relu(W_head @ x), g = W_tail @ h ----
        with tc.tile_pool(name="psmm", bufs=2, space="PSUM") as psmm:
            for j in range(2):
                ps = psmm.tile([128, 2048], F32, tag="mm")
                for q in range(4):
                    c0 = j * 2048 + q * 512
                    nc.tensor.matmul(ps[:, q * 512:(q + 1) * 512], w2[:],
                                     xbt[:, c0:c0 + 512], start=True, stop=True)
                nc.scalar.activation(h3[:, j * 16:j * 16 + 16, 2:2 + W],
                                     ps[:].rearrange("p (r w) -> p r w", w=W),
                                     AT.Relu)
            for dst_c, src_c in PADS:
                nc.scalar.copy(h3[:, 0:32, dst_c:dst_c + 1],
                               h3[:, 0:32, src_c:src_c + 1])
            ps = psmm.tile([128, 2048], F32, tag="mm")
            nc.tensor.matmul(ps[:, 0:512], w2[:], xbt[:, 4096:4608],
                             start=True, stop=True)
            nc.scalar.activation(h3[:, 32:36, 2:2 + W],
                                 ps[:, 0:512].rearrange("p (r w) -> p r w", w=W),
                                 AT.Relu)
            for dst_c, src_c in PADS:
                nc.scalar.copy(h3[:, 32:36, dst_c:dst_c + 1],
                               h3[:, 32:36, src_c:src_c + 1])

            # box filter row pass (DVE): T rows 0..19 first (gates av half 0)
            T = big.tile([128, FIN], EDT)
            T3 = T.rearrange("p (r w) -> p r w", w=W)
            nc.vector.tensor_add(T3[:, 0:20, :], tap(h3, -2, -2, 20, 2),
                                 tap(h3, -2, 0, 20, 2))
            nc.vector.tensor_add(T3[:, 0:20, :], T3[:, 0:20, :],
                                 tap(h3, -2, 2, 20, 2))
            av = big.tile([128, FOUT], EDT)
            nc.vector.tensor_add(av[:, 0:HF], T[:, 0:HF],
                                 T[:, 2 * W:2 * W + HF])
            nc.vector.tensor_add(av[:, 0:HF], av[:, 0:HF],
                                 T[:, 4 * W:4 * W + HF])
            nc.vector.tensor_add(T3[:, 20:36, :], tap(h3, -2, -2, 16, 22),
                                 tap(h3, -2, 0, 16, 22))
            nc.vector.tensor_add(T3[:, 20:36, :], T3[:, 20:36, :],
                                 tap(h3, -2, 2, 16, 22))
            # col pass half 1 on Pool (needed only once half 1 starts)
            nc.gpsimd.tensor_add(av[:, HF:2 * HF], T[:, HF:2 * HF],
                                 T[:, HF + 2 * W:HF + 2 * W + HF])
            nc.gpsimd.tensor_add(av[:, HF:2 * HF], av[:, HF:2 * HF],
                                 T[:, HF + 4 * W:HF + 4 * W + HF])

            for j in range(2):
                ps = psmm.tile([128, 2048], F32, tag="mm")
                for q in range(4):
                    r0 = j * 16 + q * 4
                    nc.tensor.matmul(
                        ps[:, q * 512:(q + 1) * 512], w3[:],
                        h3[:, r0:r0 + 4, 2:2 + W], start=True, stop=True)
                nc.scalar.copy(g3[:, j * 16:j * 16 + 16, 2:2 + W],
                               ps[:].rearrange("p (r w) -> p r w", w=W))
            ps = psmm.tile([128, 2048], F32, tag="mm")
            nc.tensor.matmul(ps[:, 0:512], w3[:], h3[:, 32:36, 2:2 + W],
                             start=True, stop=True)
            nc.scalar.copy(g3[:, 32:36, 2:2 + W],
                           ps[:, 0:512].rearrange("p (r w) -> p r w", w=W))

        av3 = av.rearrange("p (r w) -> p r w", w=W)
        for dst_c, src_c in PADS:
            nc.scalar.copy(g3[:, :, dst_c:dst_c + 1],
                           g3[:, :, src_c:src_c + 1])

        # ---- k loop ----
        OFFS = [(di, dj) for di in (-2, 0, 2) for dj in (-2, 0, 2)]
        nst = cpool.tile([18, FOUT], EDT)       # n_k rows, row pair by kr
        nsq = cpool.tile([18, FOUT], EDT)

        with tc.tile_pool(name="psnk", bufs=2, space="PSUM") as psnk, \
                tc.tile_pool(name="psfa", bufs=1, space="PSUM") as psfa:

            state = {"fs": True}   # fillers use start=True until ident(0)

            def filler(facc_ps, n=1):
                """Zero-add matmuls that keep the PE executing (and thus at
                its ramped clock) across dependency waits. Harmless: before
                ident(k=0) they are discarded by its start=True; after, they
                accumulate idm.T @ 0 = 0."""
                for _ in range(n):
                    nc.tensor.matmul(facc_ps[:, 0:512], idm[:], zb[:],
                                     start=state["fs"], stop=False,
                                     skip_group_check=True)

            def cf_steps(half, nb_c, facc_ps):
                """Deferred tail for one half, column-chunked: N2 (broadcast
                via sbm matmul through the psnk ring), reciprocal, cf =
                nb_center*rcp, res = facc*cf + x, DMA out. Fed one segment per
                k of the NEXT half."""
                hs0 = half * HF
                nc.scalar.square(nsq[:, hs0:hs0 + HF], nst[:, hs0:hs0 + HF])
                rcp = rows.tile([128, HF], F32, tag="rcp", bufs=2)
                s2l = []
                for ch in range(2):
                    s2ps = psnk.tile([128, 1024], F32, tag="nk")
                    s2l.append(s2ps)
                    for q in range(2):
                        c0 = hs0 + ch * 1024 + q * 512
                        nc.tensor.matmul(s2ps[:, q * 512:(q + 1) * 512],
                                         sbm[:], nsq[:, c0:c0 + 512],
                                         start=True, stop=True)
                    nc.vector.reciprocal_approx_fast(
                        rcp[:, ch * 1024:(ch + 1) * 1024], s2ps[:])
                yield
                cfb = rows.tile([128, HF], EDT, tag="cfb", bufs=2)
                nc.vector.tensor_mul(cfb[:, 0:1024], nb_c[:, 0:1024],
                                     rcp[:, 0:1024])
                yield
                nc.vector.tensor_mul(cfb[:, 1024:2048], nb_c[:, 1024:2048],
                                     rcp[:, 1024:2048])
                yield
                for ch in range(2):
                    cs = slice(ch * 1024, (ch + 1) * 1024)
                    res = rows.tile([128, 1024], EDT, tag="res", bufs=2)
                    nc.vector.tensor_mul(res[:], facc_ps[:, cs], cfb[:, cs])
                    nc.vector.tensor_add(
                        res[:], res[:],
                        xbt[:, 2 * W + hs0 + ch * 1024:
                             2 * W + hs0 + (ch + 1) * 1024])
                    nc.sync.dma_start(y_ext[:, hs0 + ch * 1024:
                                            hs0 + (ch + 1) * 1024], res[:])
                    yield

            pending = None
            for half in range(2):
                rh = half * 16
                hs = slice(half * HF, (half + 1) * HF)
                facc_ps = psfa.tile([128, HF], F32, tag="fa")
                state["fs"] = True

                def emit_prod(k):
                    di, dj = OFFS[k]
                    prod = gpool.tile([128, HF], EDT, tag="pp", bufs=3)
                    p3 = prod.rearrange("p (r w) -> p r w", w=W)
                    nc.vector.tensor_mul(p3[:], av3[:, rh:rh + 16, :],
                                         tap(h3, di, dj, 16, 2 + rh))
                    return prod

                prods = {0: emit_prod(0), 1: emit_prod(1)}
                pks = {}
                nb3 = None
                nbc = [None]

                def emit_ident(k):
                    pk = pks.pop(k)
                    if k == 0:
                        state["fs"] = False
                    for q in range(4):
                        nc.tensor.matmul(
                            facc_ps[:, q * 512:(q + 1) * 512], idm[:],
                            pk[:, q * 512:(q + 1) * 512],
                            start=(k == 0), stop=(k == 8))

                for k, (di, dj) in enumerate(OFFS):
                    grp, j = divmod(k, 3)
                    if j == 0:
                        nb3 = gpool.tile([128, 3 * HF], EDT, tag="nb3",
                                         bufs=3)
                        if grp == 1:
                            nbc[0] = nb3[:, HF:2 * HF]
                    prod = prods.pop(k)
                    for ch in range(2):
                        pst = psnk.tile([128, 1024], F32, tag="nk")
                        for q in range(2):
                            c0 = ch * 1024 + q * 512
                            nc.tensor.matmul(
                                pst[:, q * 512:(q + 1) * 512], bo[:],
                                prod[:, c0:c0 + 512],
                                start=True, stop=True)
                        nc.scalar.copy(
                            nb3[:, j * HF + ch * 1024:
                                j * HF + (ch + 1) * 1024], pst[:])
                    filler(facc_ps, 1)
                    kr = (k - 4) % 9          # put k=4 (center) at rows 0..1
                    nc.sync.dma_start(nst[2 * kr:2 * kr + 1, hs],
                                      nb3[0:1, j * HF:(j + 1) * HF])
                    nc.sync.dma_start(nst[2 * kr + 1:2 * kr + 2, hs],
                                      nb3[64:65, j * HF:(j + 1) * HF])

                    if k + 2 < 9:
                        prods[k + 2] = emit_prod(k + 2)

                    pk = gpool.tile([128, HF], EDT, tag="pk", bufs=2)
                    p3 = pk.rearrange("p (r w) -> p r w", w=W)
                    nb33 = nb3[:, j * HF:(j + 1) * HF] \
                        .rearrange("p (r w) -> p r w", w=W)
                    nc.vector.tensor_mul(p3[:], nb33[:],
                                         tap(g3, di, dj, 16, 2 + rh))
                    pks[k] = pk
                    if k >= 1:
                        emit_ident(k - 1)
                        filler(facc_ps, 1)
                    if pending is not None:
                        next(pending, None)
                emit_ident(8)
                if pending is not None:
                    for _ in pending:
                        pass
                pending = cf_steps(half, nbc[0], facc_ps)
                next(pending, None)
            for _ in pending:
                pass

    nc.compile()
    return nc


_NC_CACHE = [None]


def _get_nc():
    if _NC_CACHE[0] is None:
        _NC_CACHE[0] = _build()
    return _NC_CACHE[0]


def _host_prep(x):
    import ml_dtypes
    B, Cc, H, Ww = x.shape
    in_maps = []
    for core in range(N_CORES):
        b, half = core // 2, core % 2
        r0 = 64 * half
        gidx = np.clip(np.arange(r0 - 2, r0 + 66), 0, H - 1)
        xs = x[b][:, gidx, :]                     # (64, 68, 128)
        packed = np.ascontiguousarray(
            np.concatenate([xs[:, 0:36], xs[:, 32:68]], axis=0))
        in_maps.append({
            "xb": packed.reshape(128, FIN).astype(ml_dtypes.bfloat16),
        })
    return in_maps


def _const_maps(W_head, W_tail):
    import ml_dtypes

    def to_edt(a):
        return a.astype(ml_dtypes.bfloat16) if EDT == BF16 else a.astype(np.float32)

    w2 = np.zeros((128, 128), np.float32)
    w2[:64, :64] = W_head.T
    w2[64:, 64:] = W_head.T
    w3 = np.zeros((128, 128), np.float32)
    w3[:64, :64] = W_tail.T
    w3[64:, 64:] = W_tail.T
    bo = np.zeros((128, 128), np.float32)
    bo[:64, :64] = 1.0 / 9.0
    bo[64:, 64:] = 1.0 / 9.0
    sb = np.zeros((18, 128), np.float32)
    sb[0::2, :64] = 1.0
    sb[1::2, 64:] = 1.0
    return {"w2": to_edt(w2), "w3": to_edt(w3), "bo": to_edt(bo),
            "sb": to_edt(sb),
            "idm": to_edt(np.eye(128, dtype=np.float32))}


def kernel(x, W_head, W_tail):
    x = np.asarray(x, np.float32)
    W_head = np.asarray(W_head, np.float32)
    W_tail = np.asarray(W_tail, np.float32)
    nc = _get_nc()
    consts = _const_maps(W_head, W_tail)
    in_maps = [{**m, **consts} for m in _host_prep(x)]
    res = run_bass_kernel_spmd(nc, in_maps, list(range(N_CORES)))
    out = np.empty_like(x)
    for core in range(N_CORES):
        b, half = core // 2, core % 2
        r0 = 64 * half
        y = res.results[core]["y"].astype(np.float32).reshape(128, ROUT, W)
        out[b, :, r0:r0 + 32, :] = y[:64]
        out[b, :, r0 + 32:r0 + 64, :] = y[64:]
    return out


# revision 28
# speedup vs baseline: 1.5569x; 1.2714x over previous
"""Trainium2 Bass kernel for nn_LowRankLayer_dilation (B=4, C=64, H=W=128).

Math: the reference's rank-3 NMF update collapses exactly (all ranks are
initialized identically), and the eps terms are negligible for this input
distribution, giving:

    h   = relu(W_head @ x)            (per-pixel channel matmul)
    g   = W_tail @ h                  (per-pixel channel matmul)
    a   = box9(h)                     (3x3 dilation-2 box sum, edge-clamped)
    n_k = sum_c (a/9)_c * h_c(p+d_k)  (9 taps, d in {-2,0,2}^2)
    out = x + (n_4 / sum_j n_j^2) * sum_k n_k * g(p+d_k)

Sharding: pure data parallel, 8 cores = (batch b, H-half). Each core gets a
68-row halo'd slice packed as 2 channel blocks on 128 partitions:
partition p = c + 64*blk, blk A = slice rows 0..35, blk B = rows 32..67.
Channel reductions/broadcasts run on the PE via block-structured 0/1
matrices. h and g keep 2 replicate-padded columns per side (row stride 132)
so every dilated tap is a strided AP view.

v2 engine assignment (vs v1): the 9 taps are processed in 3 row-groups of 3,
each group's products computed by ONE 4-D-AP DVE op (the dj axis is an
overlapping stride-2 AP dim; av is replicated via a stride-0 broadcast dim).
PSUM->SBUF broadcast copies are split Scalar/GpSimd per 1024-chunk; the box
filter is split DVE/GpSimd; relu runs as tensor_max against a broadcast zero
tile (2x DVE mode) or on Scalar; the residual add reads the bf16 input tile
directly (no separate f32 residual DMA) and the output is bf16. All small
gather/output DMAs issue from the idle sync (SP) queue to keep GpSimd free.
"""
import sys
import contextlib
import numpy as np

sys.path.insert(0, '/opt/trn_rl_repo')

import concourse.bass as bass  # noqa: E402,F401
import concourse.bacc as bacc  # noqa: E402
import concourse.tile as tile  # noqa: E402
import concourse.mybir as mybir  # noqa: E402
from concourse.bass_utils import run_bass_kernel_spmd  # noqa: E402

F32 = mybir.dt.float32
BF16 = mybir.dt.bfloat16
AT = mybir.ActivationFunctionType

N_CORES = 8
RIN = 36          # per-block input rows (with +-2 halo)
ROUT = 32         # per-block output rows
W = 128
WP = W + 4        # padded row stride for h/g
FIN = RIN * W     # 4608
FOUT = ROUT * W   # 4096
HF = 2048         # half (16 out rows) worth of pixels per partition

EDT = BF16


def _build():
    nc = bacc.Bacc("TRN2", target_bir_lowering=False, debug=False,
                   num_devices=N_CORES)
    xb_ext = nc.dram_tensor("xb", [128, FIN], EDT, kind="ExternalInput").ap()
    w2_ext = nc.dram_tensor("w2", [128, 128], EDT, kind="ExternalInput").ap()
    w3_ext = nc.dram_tensor("w3", [128, 128], EDT, kind="ExternalInput").ap()
    bo_ext = nc.dram_tensor("bo", [128, 128], EDT, kind="ExternalInput").ap()
    sb_ext = nc.dram_tensor("sb", [18, 128], EDT, kind="ExternalInput").ap()
    id_ext = nc.dram_tensor("idm", [128, 128], EDT, kind="ExternalInput").ap()
    y_ext = nc.dram_tensor("y", [128, FOUT], EDT, kind="ExternalOutput").ap()

    with tile.TileContext(nc) as tc, contextlib.ExitStack() as ctx:
        cpool = ctx.enter_context(tc.tile_pool(name="consts", bufs=1))
        big = ctx.enter_context(tc.tile_pool(name="big", bufs=1))
        gpool = ctx.enter_context(tc.tile_pool(name="grp", bufs=2))
        rows = ctx.enter_context(tc.tile_pool(name="rows", bufs=2))

        # startup: weights first (gate LDW), x chunks spread over the three
        # HWDGE queues so transfers run in parallel
        w2 = cpool.tile([128, 128], EDT)
        nc.sync.dma_start(w2[:], w2_ext[:])
        xbt = big.tile([128, FIN], EDT)
        nc.sync.dma_start(xbt[:, 0:2048], xb_ext[:, 0:2048])
        nc.scalar.dma_start(xbt[:, 2048:4096], xb_ext[:, 2048:4096])
        nc.gpsimd.dma_start(xbt[:, 4096:4608], xb_ext[:, 4096:4608])
        w3 = cpool.tile([128, 128], EDT)
        nc.sync.dma_start(w3[:], w3_ext[:])
        bo = cpool.tile([128, 128], EDT)
        nc.gpsimd.dma_start(bo[:], bo_ext[:])
        sbm = cpool.tile([18, 128], EDT)
        nc.gpsimd.dma_start(sbm[:], sb_ext[:])
        idm = cpool.tile([128, 128], EDT)
        nc.gpsimd.dma_start(idm[:], id_ext[:])

        # h/g: (RIN, WP) row layout; data at cols 2..129, replicate pads at
        # cols 0,1,130,131.
        hf = big.tile([128, RIN * WP], EDT)
        gf = big.tile([128, RIN * WP], EDT)
        h3 = hf.rearrange("p (r w) -> p r w", w=WP)
        g3 = gf.rearrange("p (r w) -> p r w", w=WP)

        def tap(t3, di, dj, nrows=ROUT, r0=2):
            rr = r0 + di
            return t3[:, rr:rr + nrows, 2 + dj:2 + dj + W]

        PADS = ((0, 2), (1, 2), (130, 129), (131, 129))

        # ---- head + tail matmuls: h = relu(W_head @ x), g = W_tail @ h ----
        with tc.tile_pool(name="psmm", bufs=2, space="PSUM") as psmm:
            for j in range(2):
                ps = psmm.tile([128, 2048], F32, tag="mm")
                for q in range(4):
                    c0 = j * 2048 + q * 512
                    nc.tensor.matmul(ps[:, q * 512:(q + 1) * 512], w2[:],
                                     xbt[:, c0:c0 + 512], start=True, stop=True)
                nc.scalar.activation(h3[:, j * 16:j * 16 + 16, 2:2 + W],
                                     ps[:].rearrange("p (r w) -> p r w", w=W),
                                     AT.Relu)
            for dst_c, src_c in PADS:
                nc.scalar.copy(h3[:, 0:32, dst_c:dst_c + 1],
                               h3[:, 0:32, src_c:src_c + 1])
            ps = psmm.tile([128, 2048], F32, tag="mm")
            nc.tensor.matmul(ps[:, 0:512], w2[:], xbt[:, 4096:4608],
                             start=True, stop=True)
            nc.scalar.activation(h3[:, 32:36, 2:2 + W],
                                 ps[:, 0:512].rearrange("p (r w) -> p r w", w=W),
                                 AT.Relu)
            for dst_c, src_c in PADS:
                nc.scalar.copy(h3[:, 32:36, dst_c:dst_c + 1],
                               h3[:, 32:36, src_c:src_c + 1])

            # box filter row pass (DVE): T rows 0..19 first (gates av half 0)
            T = big.tile([128, FIN], EDT)
            T3 = T.rearrange("p (r w) -> p r w", w=W)
            nc.vector.tensor_add(T3[:, 0:20, :], tap(h3, -2, -2, 20, 2),
                                 tap(h3, -2, 0, 20, 2))
            nc.vector.tensor_add(T3[:, 0:20, :], T3[:, 0:20, :],
                                 tap(h3, -2, 2, 20, 2))
            av = big.tile([128, FOUT], EDT)
            nc.vector.tensor_add(av[:, 0:HF], T[:, 0:HF],
                                 T[:, 2 * W:2 * W + HF])
            nc.vector.tensor_add(av[:, 0:HF], av[:, 0:HF],
                                 T[:, 4 * W:4 * W + HF])
            nc.vector.tensor_add(T3[:, 20:36, :], tap(h3, -2, -2, 16, 22),
                                 tap(h3, -2, 0, 16, 22))
            nc.vector.tensor_add(T3[:, 20:36, :], T3[:, 20:36, :],
                                 tap(h3, -2, 2, 16, 22))
            nc.vector.tensor_add(av[:, HF:2 * HF], T[:, HF:2 * HF],
                                 T[:, HF + 2 * W:HF + 2 * W + HF])
            nc.vector.tensor_add(av[:, HF:2 * HF], av[:, HF:2 * HF],
                                 T[:, HF + 4 * W:HF + 4 * W + HF])

            for j in range(2):
                ps = psmm.tile([128, 2048], F32, tag="mm")
                for q in range(4):
                    r0 = j * 16 + q * 4
                    nc.tensor.matmul(
                        ps[:, q * 512:(q + 1) * 512], w3[:],
                        h3[:, r0:r0 + 4, 2:2 + W], start=True, stop=True)
                nc.scalar.copy(g3[:, j * 16:j * 16 + 16, 2:2 + W],
                               ps[:].rearrange("p (r w) -> p r w", w=W))
            ps = psmm.tile([128, 2048], F32, tag="mm")
            nc.tensor.matmul(ps[:, 0:512], w3[:], h3[:, 32:36, 2:2 + W],
                             start=True, stop=True)
            nc.scalar.copy(g3[:, 32:36, 2:2 + W],
                           ps[:, 0:512].rearrange("p (r w) -> p r w", w=W))

        av3 = av.rearrange("p (r w) -> p r w", w=W)
        for dst_c, src_c in PADS:
            nc.scalar.copy(g3[:, :, dst_c:dst_c + 1],
                           g3[:, :, src_c:src_c + 1])

        # ---- k loop ----
        OFFS = [(di, dj) for di in (-2, 0, 2) for dj in (-2, 0, 2)]
        nst = cpool.tile([18, FOUT], EDT)       # n_k rows, row pair by kr
        nsq = cpool.tile([18, FOUT], EDT)

        with tc.tile_pool(name="psnk", bufs=2, space="PSUM") as psnk, \
                tc.tile_pool(name="psfa", bufs=1, space="PSUM") as psfa:

            def cf_steps(half, nb_c, facc_ps):
                """Deferred tail for one half, column-chunked: N2 (broadcast
                via sbm matmul through the psnk ring), reciprocal, cf =
                nb_center*rcp, res = facc*cf + x, DMA out. Fed one segment per
                k of the NEXT half."""
                hs0 = half * HF
                nc.scalar.square(nsq[:, hs0:hs0 + HF], nst[:, hs0:hs0 + HF])
                rcp = rows.tile([128, HF], F32, tag="rcp", bufs=2)
                s2l = []
                for ch in range(2):
                    s2ps = psnk.tile([128, 1024], F32, tag="nk")
                    s2l.append(s2ps)
                    for q in range(2):
                        c0 = hs0 + ch * 1024 + q * 512
                        nc.tensor.matmul(s2ps[:, q * 512:(q + 1) * 512],
                                         sbm[:], nsq[:, c0:c0 + 512],
                                         start=True, stop=True)
                    nc.vector.reciprocal_approx_fast(
                        rcp[:, ch * 1024:(ch + 1) * 1024], s2ps[:])
                yield
                cfb = rows.tile([128, HF], EDT, tag="cfb", bufs=2)
                nc.vector.tensor_mul(cfb[:, 0:1024], nb_c[:, 0:1024],
                                     rcp[:, 0:1024])
                yield
                nc.vector.tensor_mul(cfb[:, 1024:2048], nb_c[:, 1024:2048],
                                     rcp[:, 1024:2048])
                yield
                for ch in range(2):
                    cs = slice(ch * 1024, (ch + 1) * 1024)
                    res = rows.tile([128, 1024], EDT, tag="res", bufs=2)
                    nc.vector.tensor_mul(res[:], facc_ps[:, cs], cfb[:, cs])
                    nc.vector.tensor_add(
                        res[:], res[:],
                        xbt[:, 2 * W + hs0 + ch * 1024:
                             2 * W + hs0 + (ch + 1) * 1024])
                    nc.sync.dma_start(y_ext[:, hs0 + ch * 1024:
                                            hs0 + (ch + 1) * 1024], res[:])
                    yield

            pending = None
            for half in range(2):
                rh = half * 16
                hs = slice(half * HF, (half + 1) * HF)
                facc_ps = psfa.tile([128, HF], F32, tag="fa")

                def emit_prod(k):
                    di, dj = OFFS[k]
                    prod = gpool.tile([128, HF], EDT, tag="pp", bufs=3)
                    p3 = prod.rearrange("p (r w) -> p r w", w=W)
                    nc.vector.tensor_mul(p3[:], av3[:, rh:rh + 16, :],
                                         tap(h3, di, dj, 16, 2 + rh))
                    return prod

                prods = {0: emit_prod(0), 1: emit_prod(1)}
                pks = {}
                nb3 = None
                nbc = [None]

                def emit_ident(k):
                    pk = pks.pop(k)
                    for q in range(4):
                        nc.tensor.matmul(
                            facc_ps[:, q * 512:(q + 1) * 512], idm[:],
                            pk[:, q * 512:(q + 1) * 512],
                            start=(k == 0), stop=(k == 8))

                for k, (di, dj) in enumerate(OFFS):
                    grp, j = divmod(k, 3)
                    if j == 0:
                        nb3 = gpool.tile([128, 3 * HF], EDT, tag="nb3",
                                         bufs=3)
                        if grp == 1:
                            nbc[0] = nb3[:, HF:2 * HF]
                    prod = prods.pop(k)
                    for ch in range(2):
                        pst = psnk.tile([128, 1024], F32, tag="nk")
                        for q in range(2):
                            c0 = ch * 1024 + q * 512
                            nc.tensor.matmul(
                                pst[:, q * 512:(q + 1) * 512], bo[:],
                                prod[:, c0:c0 + 512],
                                start=True, stop=True)
                        nc.scalar.copy(
                            nb3[:, j * HF + ch * 1024:
                                j * HF + (ch + 1) * 1024], pst[:])
                    kr = (k - 4) % 9          # put k=4 (center) at rows 0..1
                    nc.sync.dma_start(nst[2 * kr:2 * kr + 1, hs],
                                      nb3[0:1, j * HF:(j + 1) * HF])
                    nc.sync.dma_start(nst[2 * kr + 1:2 * kr + 2, hs],
                                      nb3[64:65, j * HF:(j + 1) * HF])

                    if k + 2 < 9:
                        prods[k + 2] = emit_prod(k + 2)

                    pk = gpool.tile([128, HF], EDT, tag="pk", bufs=2)
                    p3 = pk.rearrange("p (r w) -> p r w", w=W)
                    nb33 = nb3[:, j * HF:(j + 1) * HF] \
                        .rearrange("p (r w) -> p r w", w=W)
                    nc.vector.tensor_mul(p3[:], nb33[:],
                                         tap(g3, di, dj, 16, 2 + rh))
                    pks[k] = pk
                    if k >= 1:
                        emit_ident(k - 1)
                    if pending is not None:
                        next(pending, None)
                emit_ident(8)
                if pending is not None:
                    for _ in pending:
                        pass
                pending = cf_steps(half, nbc[0], facc_ps)
            for _ in pending:
                pass

    nc.compile()
    return nc


_NC_CACHE = [None]


def _get_nc():
    if _NC_CACHE[0] is None:
        _NC_CACHE[0] = _build()
    return _NC_CACHE[0]


def _host_prep(x):
    import ml_dtypes
    B, Cc, H, Ww = x.shape
    in_maps = []
    for core in range(N_CORES):
        b, half = core // 2, core % 2
        r0 = 64 * half
        gidx = np.clip(np.arange(r0 - 2, r0 + 66), 0, H - 1)
        xs = x[b][:, gidx, :]                     # (64, 68, 128)
        packed = np.ascontiguousarray(
            np.concatenate([xs[:, 0:36], xs[:, 32:68]], axis=0))
        in_maps.append({
            "xb": packed.reshape(128, FIN).astype(ml_dtypes.bfloat16),
        })
    return in_maps


def _const_maps(W_head, W_tail):
    import ml_dtypes

    def to_edt(a):
        return a.astype(ml_dtypes.bfloat16) if EDT == BF16 else a.astype(np.float32)

    w2 = np.zeros((128, 128), np.float32)
    w2[:64, :64] = W_head.T
    w2[64:, 64:] = W_head.T
    w3 = np.zeros((128, 128), np.float32)
    w3[:64, :64] = W_tail.T
    w3[64:, 64:] = W_tail.T
    bo = np.zeros((128, 128), np.float32)
    bo[:64, :64] = 1.0 / 9.0
    bo[64:, 64:] = 1.0 / 9.0
    sb = np.zeros((18, 128), np.float32)
    sb[0::2, :64] = 1.0
    sb[1::2, 64:] = 1.0
    return {"w2": to_edt(w2), "w3": to_edt(w3), "bo": to_edt(bo),
            "sb": to_edt(sb),
            "idm": to_edt(np.eye(128, dtype=np.float32))}


def kernel(x, W_head, W_tail):
    x = np.asarray(x, np.float32)
    W_head = np.asarray(W_head, np.float32)
    W_tail = np.asarray(W_tail, np.float32)
    nc = _get_nc()
    consts = _const_maps(W_head, W_tail)
    in_maps = [{**m, **consts} for m in _host_prep(x)]
    res = run_bass_kernel_spmd(nc, in_maps, list(range(N_CORES)))
    out = np.empty_like(x)
    for core in range(N_CORES):
        b, half = core // 2, core % 2
        r0 = 64 * half
        y = res.results[core]["y"].astype(np.float32).reshape(128, ROUT, W)
        out[b, :, r0:r0 + 32, :] = y[:64]
        out[b, :, r0 + 32:r0 + 64, :] = y[64:]
    return out
